# revision 1
# baseline (speedup 1.0000x reference)
"""Trainium2 Bass kernel for nn_Net_20512763805724 (dense_mlp, 3-layer SLP net).

Math (per layer, input p [B,L], weight w [O,L]):
    wb = sign(w)  (w>=0 -> +1 else -1)
    e  = 2p-1 ; d = 4p(1-p)
    out = (sum(d+e^2) + (e@wb.T)^2 - (e^2)@(wb^2).T) / L^2
Since d+e^2 == 1 exactly and wb^2 == 1:
    out[b,o] = (C[b] + s1[b,o]^2) / L^2
    s1 = 2*(p@wb.T) - c[o],  c[o] = sum_f wb[o,f]
    C[b] = 4*sum_f p(1-p)

Sharding: pure data parallel over batch, 8 cores x 8192 rows.  The host
pre-tiles + transposes each core's x-shard (feature-major, bf16) so on-chip
everything stays in "transposed" layout (features on SBUF partitions, batch
on the free dim): no on-chip transposes.  Output is produced as outT
[4, 8192] per core and un-transposed on the host.

Per 512-batch tile (bf16 data, fp32 PSUM):
  - mm1 on TensorE: psumA = wb1T.T @ xT   (6 k-chunks)
  - q = p(1-p)-flavoured values via one DVE scalar_tensor_tensor (3 chunks)
    and one ACT Square (3 chunks), streamed through an all-(-s) ones matmul
    -> psumB = C/L^2 broadcast across output partitions
  - ACT: t = Square(psumA*(2/L) - c/L) = s1^2/L^2
  - DVE: next_pT = (t + K) + psumB
Layer-1/2/3 psums are partition-stacked (2 tiles -> [128,512] for L1/L2,
4 tiles for L3) via matmul tile_position so elementwise ops use all 128
partitions.
"""

import sys

if "/opt/trn_rl_repo" not in sys.path:
    sys.path.insert(0, "/opt/trn_rl_repo")

import ml_dtypes
import numpy as np

BF16 = ml_dtypes.bfloat16

B = 65536
IN_DIM = 768
NCORES = 8
BC = B // NCORES            # 8192 rows per core
TILE = 512                  # batch tile (max fp32 PSUM free dim)
NT = BC // TILE             # 16 tiles per core
NCHUNK = IN_DIM // 128      # 6 feature chunks for layer 1
L1, O1 = 768, 64
L2, O2 = 64, 32
L3, O3 = 32, 4
N_DVE_CHUNKS = 3            # layer-1 q-chunks 0..2 on DVE; 3..5 on ACT
N_ACT_CHUNKS = NCHUNK - N_DVE_CHUNKS
# C-term scale: the all-ones lhsT holds -s in bf16, so use the bf16-rounded
# value as the effective scale everywhere it must stay consistent.
S1_EFF = float(np.float32(BF16(4.0 / (L1 * L1))))
S2 = 4.0 / (L2 * L2)        # 2^-10, exact in bf16
S3 = 4.0 / (L3 * L3)        # 2^-8, exact in bf16
# ACT chunks compute (x-0.5)^2 instead of (x^2-x); each contributes an extra
# -s*0.25*128 into psumB that we add back as a constant in the combine.
K1 = S1_EFF * 0.25 * 128 * N_ACT_CHUNKS

_CACHE = {}


def _build(reps=1, mode="full"):
    import contextlib

    import concourse.bacc as bacc
    import concourse.mybir as mybir
    import concourse.tile as tile

    f32 = mybir.dt.float32
    bf16 = mybir.dt.bfloat16
    AOP = mybir.AluOpType
    Square = mybir.ActivationFunctionType.Square

    nc = bacc.Bacc(None, target_bir_lowering=False)

    xt = nc.declare_dram_parameter("xt", [NT, 128, NCHUNK, TILE], bf16,
                                   isOutput=False)
    w1t = nc.declare_dram_parameter("w1t", [L1, O1], f32, isOutput=False)
    w2t = nc.declare_dram_parameter("w2t", [L2, O2], f32, isOutput=False)
    w3t = nc.declare_dram_parameter("w3t", [L3, O3], f32, isOutput=False)
    outt = nc.declare_dram_parameter("outt", [O3, BC], f32, isOutput=True)

    with tile.TileContext(nc) as tc:
        with (
            tc.tile_pool(name="const", bufs=1) as cpool,
            tc.tile_pool(name="xp", bufs=6) as xpool,
            tc.tile_pool(name="qp", bufs=4) as qpool,
            tc.tile_pool(name="sb", bufs=3) as spool,
            tc.tile_pool(name="psA", bufs=2, space="PSUM") as pA,
            tc.tile_pool(name="psB", bufs=2, space="PSUM") as pB,
            tc.tile_pool(name="psS", bufs=1, space="PSUM") as pS,
        ):
            # ---------------- weight prep (one-time) ----------------
            w1raw = cpool.tile([128, NCHUNK, O1], f32)
            nc.sync.dma_start(
                out=w1raw, in_=w1t[:].rearrange("(c p) o -> p c o", p=128)
            )
            lhsT1 = cpool.tile([128, NCHUNK, O1], bf16)
            # binarize: (w>=0)*2-1 -> {-1,+1}
            nc.vector.tensor_scalar(lhsT1, w1raw, 0.0, 2.0, AOP.is_ge, AOP.mult)
            nc.vector.tensor_scalar(lhsT1, lhsT1, 1.0, None, AOP.subtract)

            w2raw = cpool.tile([L2, O2], f32)
            nc.sync.dma_start(out=w2raw, in_=w2t[:])
            lhsT2 = cpool.tile([128, O2], bf16)
            nc.vector.tensor_scalar(
                lhsT2[0:L2, :], w2raw, 0.0, 2.0, AOP.is_ge, AOP.mult
            )
            nc.vector.tensor_scalar(
                lhsT2[0:L2, :], lhsT2[0:L2, :], 1.0, None, AOP.subtract
            )
            nc.sync.dma_start(out=lhsT2[L2:128, :], in_=lhsT2[0:L2, :])

            w3raw = cpool.tile([L3, O3], f32)
            nc.sync.dma_start(out=w3raw, in_=w3t[:])
            lhsT3 = cpool.tile([128, O3], bf16)
            nc.vector.tensor_scalar(
                lhsT3[0:L3, :], w3raw, 0.0, 2.0, AOP.is_ge, AOP.mult
            )
            nc.vector.tensor_scalar(
                lhsT3[0:L3, :], lhsT3[0:L3, :], 1.0, None, AOP.subtract
            )
            for j in range(1, 4):
                nc.sync.dma_start(
                    out=lhsT3[32 * j : 32 * j + L3, :], in_=lhsT3[0:L3, :]
                )

            onesB1 = cpool.tile([128, O1], bf16)
            nc.vector.memset(onesB1, -S1_EFF)
            onesB2 = cpool.tile([128, O2], bf16)
            nc.vector.memset(onesB2, -S2)
            onesB3 = cpool.tile([128, O3], bf16)
            nc.vector.memset(onesB3, -S3)
            ones_col = cpool.tile([128, 8], bf16)
            nc.vector.memset(ones_col, 1.0)
            biasq = cpool.tile([128, 1], f32)
            nc.vector.memset(biasq, -0.5)

            # c[o] = sum_f wb[o,f] via tiny ones matmuls; bias = -c/L,
            # replicated to every stacked position.
            bias1 = cpool.tile([128, 1], f32)
            c1ps = pS.tile([O1, 8], f32, tag="A3")
            for c in range(NCHUNK):
                nc.tensor.matmul(
                    c1ps,
                    lhsT1[:, c, :],
                    ones_col,
                    start=(c == 0),
                    stop=(c == NCHUNK - 1),
                )
            nc.vector.tensor_scalar(
                bias1[0:O1, :], c1ps[:, 0:1], -1.0 / L1, None, AOP.mult
            )
            nc.sync.dma_start(out=bias1[O1:128, :], in_=bias1[0:O1, :])

            bias2 = cpool.tile([128, 1], f32)
            c2ps = pS.tile([O2, 8], f32, tag="A3")
            nc.tensor.matmul(c2ps, lhsT2[0:L2, :], ones_col[0:L2, :])
            nc.vector.tensor_scalar(
                bias2[0:O2, :], c2ps[:, 0:1], -1.0 / L2, None, AOP.mult
            )
            for j in range(1, 4):
                nc.sync.dma_start(
                    out=bias2[32 * j : 32 * j + O2, :], in_=bias2[0:O2, :]
                )

            bias3 = cpool.tile([128, 1], f32)
            c3ps = pS.tile([O3, 8], f32, tag="A3")
            nc.tensor.matmul(c3ps, lhsT3[0:L3, :], ones_col[0:L3, :])
            nc.vector.tensor_scalar(
                bias3[0:O3, :], c3ps[:, 0:1], -1.0 / L3, None, AOP.mult
            )
            for j in range(1, 4):
                nc.sync.dma_start(
                    out=bias3[32 * j : 32 * j + O3, :], in_=bias3[0:O3, :]
                )

            # ---------------- main loop ----------------
            loop_cm = (
                tc.For_i(0, reps, 1) if reps > 1 else contextlib.nullcontext()
            )
            with loop_cm:
              if mode == "dma4":
                  for g in range(NT // 4):
                      xg = xpool.tile([128, 4, NCHUNK, TILE], bf16, tag="xg",
                                      name=f"xg_{g}", bufs=2)
                      nc.sync.dma_start(
                          out=xg,
                          in_=xt[4 * g : 4 * g + 4].rearrange(
                              "t p c b -> p t c b"
                          ),
                      )
                      sink = spool.tile([128, 8], f32, tag="sink",
                                        name=f"sinkg_{g}")
                      nc.vector.scalar_tensor_tensor(
                          sink, xg[:, 0, 0, 0:8], 1.0, xg[:, 1, 0, 0:8],
                          AOP.mult, AOP.add,
                      )
                      nc.sync.dma_start(
                          out=outt[:, g * TILE : g * TILE + 8],
                          in_=sink[0:4, :],
                      )
              psA1 = psB1 = p2p = q2p = psA2 = psB2 = None
              for t in range(NT if mode != "dma4" else 0):
                pj = t % 2   # position within a pair
                qj = t % 4   # position within a quad

                x_t = xpool.tile([128, NCHUNK, TILE], bf16, tag="x",
                                 name=f"x_{t}")
                nc.sync.dma_start(out=x_t, in_=xt[t])

                if mode == "dma":
                    sink = spool.tile([128, 8], f32, tag="sink",
                                      name=f"sink_{t}")
                    nc.vector.scalar_tensor_tensor(
                        sink, x_t[:, 0, 0:8], 1.0, x_t[:, 1, 0:8],
                        AOP.mult, AOP.add,
                    )
                    nc.scalar.dma_start(
                        out=outt[:, t * TILE : t * TILE + 8], in_=sink[0:4, :]
                    )
                    continue

                if pj == 0:
                    psA1 = pA.tile([128, TILE], f32, tag="A1", name=f"psA1_{t}")
                    psB1 = pB.tile([128, TILE], f32, tag="B1", name=f"psB1_{t}")
                rowA = psA1[O1 * pj : O1 * (pj + 1), :]
                rowB = psB1[O1 * pj : O1 * (pj + 1), :]

                # layer-1 matmul: psumA = wbT.T @ x  (accumulate 6 chunks)
                for c in range(NCHUNK):
                    nc.tensor.matmul(
                        rowA,
                        lhsT1[:, c, :],
                        x_t[:, c, :],
                        start=(c == 0),
                        stop=(c == NCHUNK - 1),
                        tile_position=(0, O1 * pj),
                    )

                if mode == "mm1":
                    if pj == 1:
                        t2p = spool.tile([128, TILE], f32, tag="t2",
                                         name=f"t2_{t}")
                        nc.scalar.activation(
                            t2p, psA1, Square, bias=bias1, scale=2.0 / L1
                        )
                        nc.scalar.dma_start(
                            out=outt[:, t * TILE : (t + 1) * TILE],
                            in_=t2p[0:4, :],
                        )
                    continue

                # q-chunks: DVE (x-1)*x on chunks 0..2; ACT (x-0.5)^2 on 3..5
                q_t = qpool.tile([128, NCHUNK, TILE], bf16, tag="q",
                                 name=f"q_{t}")
                nc.vector.scalar_tensor_tensor(
                    q_t[:, 0:N_DVE_CHUNKS, :],
                    x_t[:, 0:N_DVE_CHUNKS, :],
                    1.0,
                    x_t[:, 0:N_DVE_CHUNKS, :],
                    AOP.subtract,
                    AOP.mult,
                )
                nc.scalar.activation(
                    q_t[:, N_DVE_CHUNKS:, :],
                    x_t[:, N_DVE_CHUNKS:, :],
                    Square,
                    bias=biasq,
                    scale=1.0,
                )

                # C-term: psumB = (-s)*ones.T @ q -> C/L^2 on all 64 rows
                for c in range(NCHUNK):
                    nc.tensor.matmul(
                        rowB,
                        onesB1,
                        q_t[:, c, :],
                        start=(c == 0),
                        stop=(c == NCHUNK - 1),
                        tile_position=(0, O1 * pj),
                    )

                if pj == 1:
                    # layer-1 combine for the pair -> p2 [128, 512]
                    t2p = spool.tile([128, TILE], f32, tag="t2", name=f"t2_{t}")
                    nc.scalar.activation(
                        t2p, psA1, Square, bias=bias1, scale=2.0 / L1
                    )
                    p2p = spool.tile([128, TILE], bf16, tag="p2", name=f"p2_{t}")
                    nc.vector.scalar_tensor_tensor(
                        p2p, t2p, K1, psB1, AOP.add, AOP.add
                    )
                    # layer 2 for both tiles of the pair
                    q2p = spool.tile([128, TILE], bf16, tag="q2", name=f"q2_{t}")
                    nc.vector.scalar_tensor_tensor(
                        q2p, p2p, 1.0, p2p, AOP.subtract, AOP.mult
                    )
                    if qj == 1:
                        psA2 = pA.tile([128, TILE], f32, tag="A2", bufs=1,
                                       name=f"psA2_{t}")
                        psB2 = pB.tile([128, TILE], f32, tag="B2", bufs=1,
                                       name=f"psB2_{t}")
                    for half, tq in ((0, qj - 1), (1, qj)):
                        hs = slice(O1 * half, O1 * (half + 1))
                        tp = (O1 * half, O2 * tq)
                        nc.tensor.matmul(
                            psA2[O2 * tq : O2 * (tq + 1), :],
                            lhsT2[hs, :],
                            p2p[hs, :],
                            tile_position=tp,
                        )
                        nc.tensor.matmul(
                            psB2[O2 * tq : O2 * (tq + 1), :],
                            onesB2[hs, :],
                            q2p[hs, :],
                            tile_position=tp,
                        )

                if qj == 3:
                    # layer-2 combine for the quad -> p3 [128, 512]
                    tq2 = spool.tile([128, TILE], f32, tag="tq2", name=f"tq2_{t}")
                    nc.scalar.activation(
                        tq2, psA2, Square, bias=bias2, scale=2.0 / L2
                    )
                    p3q = spool.tile([128, TILE], bf16, tag="p3", name=f"p3_{t}")
                    nc.vector.scalar_tensor_tensor(
                        p3q, tq2, 0.0, psB2, AOP.add, AOP.add
                    )
                    # layer 3
                    q3q = spool.tile([128, TILE], bf16, tag="q3", name=f"q3_{t}")
                    nc.vector.scalar_tensor_tensor(
                        q3q, p3q, 1.0, p3q, AOP.subtract, AOP.mult
                    )
                    psA3 = pS.tile([128, TILE], f32, tag="A3", name=f"psA3_{t}")
                    psB3 = pS.tile([128, TILE], f32, tag="B3", name=f"psB3_{t}")
                    for j in range(4):
                        js = slice(32 * j, 32 * j + L3)
                        tp = (32 * j, 32 * j)
                        nc.tensor.matmul(
                            psA3[32 * j : 32 * j + O3, :],
                            lhsT3[js, :],
                            p3q[js, :],
                            tile_position=tp,
                        )
                        nc.tensor.matmul(
                            psB3[32 * j : 32 * j + O3, :],
                            onesB3[js, :],
                            q3q[js, :],
                            tile_position=tp,
                        )
                    t3q = spool.tile([128, TILE], f32, tag="t3", name=f"t3_{t}")
                    nc.scalar.activation(
                        t3q, psA3, Square, bias=bias3, scale=2.0 / L3
                    )
                    outq = spool.tile([128, TILE], f32, tag="outq",
                                      name=f"outq_{t}")
                    nc.vector.scalar_tensor_tensor(
                        outq, t3q, 0.0, psB3, AOP.add, AOP.add
                    )
                    for j in range(4):
                        tt = t - 3 + j
                        nc.scalar.dma_start(
                            out=outt[:, tt * TILE : (tt + 1) * TILE],
                            in_=outq[32 * j : 32 * j + O3, :],
                        )

    nc.compile()
    return nc


def _get_nc(reps=1, mode="full"):
    key = ("nc", reps, mode)
    if key not in _CACHE:
        _CACHE[key] = _build(reps, mode)
    return _CACHE[key]


def _make_in_maps(x, w1, w2, w3):
    x = np.asarray(x, dtype=np.float32)
    w1t = np.ascontiguousarray(np.asarray(w1, dtype=np.float32).T)
    w2t = np.ascontiguousarray(np.asarray(w2, dtype=np.float32).T)
    w3t = np.ascontiguousarray(np.asarray(w3, dtype=np.float32).T)
    xs = x.reshape(NCORES, NT, TILE, NCHUNK, 128)
    # [core][tile, partition(f%128), chunk(f//128), batch-in-tile], bf16
    xtiled = np.ascontiguousarray(
        xs.transpose(0, 1, 4, 3, 2).astype(BF16)
    )
    return [
        {"xt": xtiled[i], "w1t": w1t, "w2t": w2t, "w3t": w3t}
        for i in range(NCORES)
    ]


def kernel(x, w1, w2, w3):
    from concourse.bass_utils import run_bass_kernel_spmd

    nc = _get_nc()
    in_maps = _make_in_maps(x, w1, w2, w3)
    res = run_bass_kernel_spmd(nc, in_maps, core_ids=list(range(NCORES)))
    return np.concatenate(
        [res.results[i]["outt"].T for i in range(NCORES)], axis=0
    ).astype(np.float32)


def bench(x, w1, w2, w3, iters=20, reps=1, mode="full", cores=NCORES):
    """Time device execution with a persistent jit and device-resident
    inputs (excludes host<->device transfer and compile).  Returns
    (output, per_call_seconds_list)."""
    import time

    import jax
    from jax.sharding import Mesh, NamedSharding, PartitionSpec
    from jax.experimental.shard_map import shard_map

    import concourse.mybir as mybir
    from concourse import bass2jax
    from concourse.bass2jax import _bass_exec_p, install_neuronx_cc_hook

    nc = _get_nc(reps, mode)
    install_neuronx_cc_hook()
    in_maps = _make_in_maps(x, w1, w2, w3)

    partition_name = (
        nc.partition_id_tensor.name if nc.partition_id_tensor else None
    )
    in_names, out_names, out_avals, zero_outs = [], [], [], []
    for alloc in nc.m.functions[0].allocations:
        if not isinstance(alloc, mybir.MemoryLocationSet):
            continue
        name = alloc.memorylocations[0].name
        if alloc.kind == "ExternalInput":
            if name != partition_name:
                in_names.append(name)
        elif alloc.kind == "ExternalOutput":
            out_names.append(name)
            shape = tuple(alloc.tensor_shape)
            dtype = mybir.dt.np(alloc.dtype)
            out_avals.append(jax.core.ShapedArray(shape, dtype))
            zero_outs.append(np.zeros(shape, dtype))
    n_params = len(in_names)
    in_names = in_names + out_names
    if partition_name is not None:
        in_names = in_names + [partition_name]

    def _body(*args):
        operands = list(args)
        if partition_name is not None:
            operands.append(bass2jax.partition_id_tensor())
        outs = _bass_exec_p.bind(
            *operands,
            out_avals=tuple(out_avals),
            in_names=tuple(in_names),
            out_names=tuple(out_names),
            lowering_input_output_aliases=(),
            sim_require_finite=True,
            sim_require_nnan=True,
            nc=nc,
        )
        return tuple(outs)

    devices = jax.devices()[:cores]
    mesh = Mesh(np.asarray(devices), ("core",))
    in_specs = (PartitionSpec("core"),) * (n_params + len(out_names))
    out_specs = (PartitionSpec("core"),) * len(out_names)
    fn = jax.jit(
        shard_map(_body, mesh=mesh, in_specs=in_specs, out_specs=out_specs,
                  check_rep=False),
        keep_unused=True,
    )
    sh = NamedSharding(mesh, PartitionSpec("core"))
    dev_in = [
        jax.device_put(
            np.concatenate([in_maps[c][nm] for c in range(cores)], axis=0), sh
        )
        for nm in in_names[:n_params]
    ]
    dev_zero = [
        jax.device_put(
            np.zeros((cores * z.shape[0], *z.shape[1:]), z.dtype), sh
        )
        for z in zero_outs
    ]
    out = fn(*dev_in, *dev_zero)
    jax.block_until_ready(out)
    times = []
    for _ in range(iters):
        t0 = time.perf_counter()
        out = fn(*dev_in, *dev_zero)
        jax.block_until_ready(out)
        times.append(time.perf_counter() - t0)
    out_np = np.asarray(out[0]).reshape(cores, *out_avals[0].shape)
    result = np.concatenate([out_np[c].T for c in range(cores)], axis=0)
    return result.astype(np.float32), times



# revision 2
# speedup vs baseline: 1049.9396x; 1049.9396x over previous
"""Trainium2 Bass kernel for nn_Net_20512763805724 (dense_mlp, 3-layer SLP net).

Math (per layer, input p [B,L], weight w [O,L]):
    wb = sign(w)  (w>=0 -> +1 else -1)
    e  = 2p-1 ; d = 4p(1-p)
    out = (sum(d+e^2) + (e@wb.T)^2 - (e^2)@(wb^2).T) / L^2
Since d+e^2 == 1 exactly and wb^2 == 1:
    out[b,o] = (C[b] + s1[b,o]^2) / L^2
    s1 = 2*(p@wb.T) - c[o],  c[o] = sum_f wb[o,f]
    C[b] = 4*sum_f p(1-p)

Sharding: pure data parallel over batch, 8 cores x 8192 rows.  The host
pre-tiles + transposes each core's x-shard (feature-major, bf16) so on-chip
everything stays in "transposed" layout (features on SBUF partitions, batch
on the free dim): no on-chip transposes.  Output is produced as outT
[4, 8192] per core and un-transposed on the host.

All weight preparation (binarize, column sums -> biases, partition
replication) is done on the host: the device program starts streaming x
immediately (x DMAs on the sync queue; tiny weight DMAs on the gpsimd
queue so they cannot head-of-line block x).

Per 512-batch tile (bf16 data, fp32 PSUM):
  - mm1 on TensorE: psumA = wb1T.T @ xT   (6 k-chunks)
  - q-values: chunks 0..3 on DVE as one flat scalar_tensor_tensor
    ((x-1)*x = -q), chunks 4..5 on ACT as one Square ((x-0.5)^2 = 0.25-q);
    streamed through an all-(-s) ones matmul -> psumB ~ C/L^2 broadcast
  - ACT: t = Square(psumA*(2/L) + bias) = s1^2/L^2
  - DVE: next_pT = (t + K) + psumB
Layer-1/2/3 psums are partition-stacked (2 tiles -> [128,512] for L1/L2,
4 tiles for L3) via matmul tile_position so elementwise ops use all 128
partitions.  Output DMAs ride the gpsimd queue (idle engine).
"""

import sys

if "/opt/trn_rl_repo" not in sys.path:
    sys.path.insert(0, "/opt/trn_rl_repo")

import ml_dtypes
import numpy as np

BF16 = ml_dtypes.bfloat16

B = 65536
IN_DIM = 768
NCORES = 8
BC = B // NCORES            # 8192 rows per core
TILE = 512                  # batch tile (max fp32 PSUM free dim)
NT = BC // TILE             # 16 tiles per core
NCHUNK = IN_DIM // 128      # 6 feature chunks for layer 1
L1, O1 = 768, 64
L2, O2 = 64, 32
L3, O3 = 32, 4
N_DVE_CHUNKS = 4            # layer-1 q-chunks 0..3 on DVE; 4..5 on ACT
N_ACT_CHUNKS = NCHUNK - N_DVE_CHUNKS
XBUFS = 10                  # x-tile prefetch depth
# C-term scale: the all-ones lhsT holds -s in bf16, so use the bf16-rounded
# value as the effective scale everywhere it must stay consistent.
S1_EFF = float(np.float32(BF16(4.0 / (L1 * L1))))
S2 = 4.0 / (L2 * L2)        # 2^-10, exact in bf16
S3 = 4.0 / (L3 * L3)        # 2^-8, exact in bf16
# ACT chunks compute (x-0.5)^2 instead of (x^2-x); each contributes an extra
# -s*0.25*128 into psumB that we add back as a constant in the combine.
K1 = S1_EFF * 0.25 * 128 * N_ACT_CHUNKS

_CACHE = {}


def _build(reps=1):
    import contextlib

    import concourse.bacc as bacc
    import concourse.mybir as mybir
    import concourse.tile as tile

    f32 = mybir.dt.float32
    bf16 = mybir.dt.bfloat16
    AOP = mybir.AluOpType
    Square = mybir.ActivationFunctionType.Square

    nc = bacc.Bacc(None, target_bir_lowering=False)

    xt = nc.declare_dram_parameter("xt", [NT, 128, NCHUNK * TILE], bf16,
                                   isOutput=False)
    w1d = nc.declare_dram_parameter("w1d", [128, NCHUNK, O1], bf16,
                                    isOutput=False)
    w2d = nc.declare_dram_parameter("w2d", [128, O2], bf16, isOutput=False)
    w3d = nc.declare_dram_parameter("w3d", [128, O3], bf16, isOutput=False)
    b1d = nc.declare_dram_parameter("b1d", [128, 1], f32, isOutput=False)
    b2d = nc.declare_dram_parameter("b2d", [128, 1], f32, isOutput=False)
    b3d = nc.declare_dram_parameter("b3d", [128, 1], f32, isOutput=False)
    outt = nc.declare_dram_parameter("outt", [O3, BC], f32, isOutput=True)

    with tile.TileContext(nc) as tc:
        with (
            tc.tile_pool(name="const", bufs=1) as cpool,
            tc.tile_pool(name="xp", bufs=XBUFS) as xpool,
            tc.tile_pool(name="qd", bufs=3) as qdpool,
            tc.tile_pool(name="qa", bufs=3) as qapool,
            tc.tile_pool(name="sb", bufs=2) as spool,
            tc.tile_pool(name="psA", bufs=2, space="PSUM") as pA,
            tc.tile_pool(name="psB", bufs=2, space="PSUM") as pB,
            tc.tile_pool(name="psS", bufs=1, space="PSUM") as pS,
        ):
            # ------------- constants (host-precomputed weights) -------------
            lhsT1 = cpool.tile([128, NCHUNK, O1], bf16)
            nc.gpsimd.dma_start(out=lhsT1, in_=w1d[:])
            lhsT2 = cpool.tile([128, O2], bf16)
            nc.gpsimd.dma_start(out=lhsT2, in_=w2d[:])
            lhsT3 = cpool.tile([128, O3], bf16)
            nc.gpsimd.dma_start(out=lhsT3, in_=w3d[:])
            bias1 = cpool.tile([128, 1], f32)
            nc.gpsimd.dma_start(out=bias1, in_=b1d[:])
            bias2 = cpool.tile([128, 1], f32)
            nc.gpsimd.dma_start(out=bias2, in_=b2d[:])
            bias3 = cpool.tile([128, 1], f32)
            nc.gpsimd.dma_start(out=bias3, in_=b3d[:])

            onesB1 = cpool.tile([128, O1], bf16)
            nc.vector.memset(onesB1, -S1_EFF)
            onesB2 = cpool.tile([128, O2], bf16)
            nc.vector.memset(onesB2, -S2)
            onesB3 = cpool.tile([128, O3], bf16)
            nc.vector.memset(onesB3, -S3)
            biasq = cpool.tile([128, 1], f32)
            nc.vector.memset(biasq, -0.5)

            # ---------------- main loop ----------------
            loop_cm = (
                tc.For_i(0, reps, 1) if reps > 1 else contextlib.nullcontext()
            )
            with loop_cm:
              psA1 = psB1 = p2p = q2p = psA2 = psB2 = None
              for t in range(NT):
                pj = t % 2   # position within a pair
                qj = t % 4   # position within a quad

                x_t = xpool.tile([128, NCHUNK * TILE], bf16, tag="x",
                                 name=f"x_{t}")
                nc.sync.dma_start(out=x_t, in_=xt[t])

                if pj == 0:
                    psA1 = pA.tile([128, TILE], f32, tag="A1", name=f"psA1_{t}")
                    psB1 = pB.tile([128, TILE], f32, tag="B1", name=f"psB1_{t}")
                rowA = psA1[O1 * pj : O1 * (pj + 1), :]
                rowB = psB1[O1 * pj : O1 * (pj + 1), :]

                # layer-1 matmul: psumA = wbT.T @ x  (accumulate 6 chunks)
                for c in range(NCHUNK):
                    nc.tensor.matmul(
                        rowA,
                        lhsT1[:, c, :],
                        x_t[:, c * TILE : (c + 1) * TILE],
                        start=(c == 0),
                        stop=(c == NCHUNK - 1),
                        tile_position=(0, O1 * pj),
                    )

                # q-chunks: DVE (x-1)*x on chunks 0..3; ACT (x-0.5)^2 on 4..5
                q_d = qdpool.tile([128, N_DVE_CHUNKS * TILE], bf16, tag="qd",
                                  name=f"qd_{t}")
                nc.vector.scalar_tensor_tensor(
                    q_d,
                    x_t[:, 0 : N_DVE_CHUNKS * TILE],
                    1.0,
                    x_t[:, 0 : N_DVE_CHUNKS * TILE],
                    AOP.subtract,
                    AOP.mult,
                )
                q_a = qapool.tile([128, N_ACT_CHUNKS * TILE], bf16, tag="qa",
                                  name=f"qa_{t}")
                nc.scalar.activation(
                    q_a,
                    x_t[:, N_DVE_CHUNKS * TILE :],
                    Square,
                    bias=biasq,
                    scale=1.0,
                )

                # C-term: psumB = (-s)*ones.T @ q -> C/L^2 on all 64 rows
                for c in range(NCHUNK):
                    if c < N_DVE_CHUNKS:
                        src = q_d[:, c * TILE : (c + 1) * TILE]
                    else:
                        cc = c - N_DVE_CHUNKS
                        src = q_a[:, cc * TILE : (cc + 1) * TILE]
                    nc.tensor.matmul(
                        rowB,
                        onesB1,
                        src,
                        start=(c == 0),
                        stop=(c == NCHUNK - 1),
                        tile_position=(0, O1 * pj),
                    )

                if pj == 1:
                    # layer-1 combine for the pair -> p2 [128, 512]
                    t2p = spool.tile([128, TILE], f32, tag="t2", name=f"t2_{t}")
                    nc.scalar.activation(
                        t2p, psA1, Square, bias=bias1, scale=2.0 / L1
                    )
                    p2p = spool.tile([128, TILE], bf16, tag="p2", name=f"p2_{t}")
                    nc.vector.scalar_tensor_tensor(
                        p2p, t2p, K1, psB1, AOP.add, AOP.add
                    )
                    # layer 2 for both tiles of the pair
                    q2p = spool.tile([128, TILE], bf16, tag="q2", name=f"q2_{t}")
                    nc.vector.scalar_tensor_tensor(
                        q2p, p2p, 1.0, p2p, AOP.subtract, AOP.mult
                    )
                    if qj == 1:
                        psA2 = pA.tile([128, TILE], f32, tag="A2", bufs=1,
                                       name=f"psA2_{t}")
                        psB2 = pB.tile([128, TILE], f32, tag="B2", bufs=1,
                                       name=f"psB2_{t}")
                    for half, tq in ((0, qj - 1), (1, qj)):
                        hs = slice(O1 * half, O1 * (half + 1))
                        tp = (O1 * half, O2 * tq)
                        nc.tensor.matmul(
                            psA2[O2 * tq : O2 * (tq + 1), :],
                            lhsT2[hs, :],
                            p2p[hs, :],
                            tile_position=tp,
                        )
                        nc.tensor.matmul(
                            psB2[O2 * tq : O2 * (tq + 1), :],
                            onesB2[hs, :],
                            q2p[hs, :],
                            tile_position=tp,
                        )

                if qj == 3:
                    # layer-2 combine for the quad -> p3 [128, 512]
                    tq2 = spool.tile([128, TILE], f32, tag="tq2", name=f"tq2_{t}")
                    nc.scalar.activation(
                        tq2, psA2, Square, bias=bias2, scale=2.0 / L2
                    )
                    p3q = spool.tile([128, TILE], bf16, tag="p3", name=f"p3_{t}")
                    nc.vector.scalar_tensor_tensor(
                        p3q, tq2, 0.0, psB2, AOP.add, AOP.add
                    )
                    # layer 3
                    q3q = spool.tile([128, TILE], bf16, tag="q3", name=f"q3_{t}")
                    nc.vector.scalar_tensor_tensor(
                        q3q, p3q, 1.0, p3q, AOP.subtract, AOP.mult
                    )
                    psA3 = pS.tile([128, TILE], f32, tag="A3", name=f"psA3_{t}")
                    psB3 = pS.tile([128, TILE], f32, tag="B3", name=f"psB3_{t}")
                    for j in range(4):
                        js = slice(32 * j, 32 * j + L3)
                        tp = (32 * j, 32 * j)
                        nc.tensor.matmul(
                            psA3[32 * j : 32 * j + O3, :],
                            lhsT3[js, :],
                            p3q[js, :],
                            tile_position=tp,
                        )
                        nc.tensor.matmul(
                            psB3[32 * j : 32 * j + O3, :],
                            onesB3[js, :],
                            q3q[js, :],
                            tile_position=tp,
                        )
                    t3q = spool.tile([128, TILE], f32, tag="t3", name=f"t3_{t}")
                    nc.scalar.activation(
                        t3q, psA3, Square, bias=bias3, scale=2.0 / L3
                    )
                    outq = spool.tile([128, TILE], f32, tag="outq",
                                      name=f"outq_{t}")
                    nc.vector.scalar_tensor_tensor(
                        outq, t3q, 0.0, psB3, AOP.add, AOP.add
                    )
                    for j in range(4):
                        tt = t - 3 + j
                        nc.gpsimd.dma_start(
                            out=outt[:, tt * TILE : (tt + 1) * TILE],
                            in_=outq[32 * j : 32 * j + O3, :],
                        )

    nc.compile()
    return nc


def _get_nc(reps=1):
    key = ("nc", reps)
    if key not in _CACHE:
        _CACHE[key] = _build(reps)
    return _CACHE[key]


def _make_in_maps(x, w1, w2, w3):
    x = np.asarray(x, dtype=np.float32)
    w1 = np.asarray(w1, dtype=np.float32)
    w2 = np.asarray(w2, dtype=np.float32)
    w3 = np.asarray(w3, dtype=np.float32)

    # binarized weights and their column sums (all tiny -> host)
    wb1 = np.where(w1 >= 0, 1.0, -1.0).astype(np.float32)   # [64, 768]
    wb2 = np.where(w2 >= 0, 1.0, -1.0).astype(np.float32)   # [32, 64]
    wb3 = np.where(w3 >= 0, 1.0, -1.0).astype(np.float32)   # [4, 32]

    # lhsT1[p, c, o] = wb1[o, c*128+p]
    w1d = np.ascontiguousarray(
        wb1.T.reshape(NCHUNK, 128, O1).transpose(1, 0, 2)
    ).astype(BF16)
    # lhsT2: [128, 32], rows 0..63 = wb2.T, rows 64..127 = copy
    w2d = np.ascontiguousarray(np.tile(wb2.T, (2, 1))).astype(BF16)
    # lhsT3: [128, 4], wb3.T replicated 4x
    w3d = np.ascontiguousarray(np.tile(wb3.T, (4, 1))).astype(BF16)

    c1 = wb1.sum(axis=1)   # [64]
    c2 = wb2.sum(axis=1)   # [32]
    c3 = wb3.sum(axis=1)   # [4]
    b1d = np.ascontiguousarray(
        np.tile(-c1 / L1, 2).reshape(128, 1)
    ).astype(np.float32)
    b2d = np.ascontiguousarray(
        np.tile(-c2 / L2, 4).reshape(128, 1)
    ).astype(np.float32)
    b3 = np.zeros((4, 32), np.float32)
    b3[:, :O3] = -c3 / L3
    b3d = np.ascontiguousarray(b3.reshape(128, 1))

    xs = x.reshape(NCORES, NT, TILE, NCHUNK, 128)
    # [core][tile, partition(f%128), chunk(f//128)*TILE + batch-in-tile], bf16
    xtiled = np.ascontiguousarray(
        xs.transpose(0, 1, 4, 3, 2).astype(BF16)
    ).reshape(NCORES, NT, 128, NCHUNK * TILE)
    return [
        {
            "xt": xtiled[i],
            "w1d": w1d,
            "w2d": w2d,
            "w3d": w3d,
            "b1d": b1d,
            "b2d": b2d,
            "b3d": b3d,
        }
        for i in range(NCORES)
    ]


def kernel(x, w1, w2, w3):
    from concourse.bass_utils import run_bass_kernel_spmd

    nc = _get_nc()
    in_maps = _make_in_maps(x, w1, w2, w3)
    res = run_bass_kernel_spmd(nc, in_maps, core_ids=list(range(NCORES)))
    return np.concatenate(
        [res.results[i]["outt"].T for i in range(NCORES)], axis=0
    ).astype(np.float32)


def bench(x, w1, w2, w3, iters=20, reps=1, cores=NCORES):
    """Time device execution with a persistent jit and device-resident
    inputs (excludes host<->device transfer and compile).  Returns
    (output, per_call_seconds_list).  NOTE: per-call wall time under axon
    is dominated by a fixed ~80ms relay dispatch latency; use the NTFF
    profile (run_bass_kernel_spmd(trace=True)) for true HW exec time."""
    import time

    import jax
    from jax.sharding import Mesh, NamedSharding, PartitionSpec
    from jax.experimental.shard_map import shard_map

    import concourse.mybir as mybir
    from concourse import bass2jax
    from concourse.bass2jax import _bass_exec_p, install_neuronx_cc_hook

    nc = _get_nc(reps)
    install_neuronx_cc_hook()
    in_maps = _make_in_maps(x, w1, w2, w3)

    partition_name = (
        nc.partition_id_tensor.name if nc.partition_id_tensor else None
    )
    in_names, out_names, out_avals, zero_outs = [], [], [], []
    for alloc in nc.m.functions[0].allocations:
        if not isinstance(alloc, mybir.MemoryLocationSet):
            continue
        name = alloc.memorylocations[0].name
        if alloc.kind == "ExternalInput":
            if name != partition_name:
                in_names.append(name)
        elif alloc.kind == "ExternalOutput":
            out_names.append(name)
            shape = tuple(alloc.tensor_shape)
            dtype = mybir.dt.np(alloc.dtype)
            out_avals.append(jax.core.ShapedArray(shape, dtype))
            zero_outs.append(np.zeros(shape, dtype))
    n_params = len(in_names)
    in_names = in_names + out_names
    if partition_name is not None:
        in_names = in_names + [partition_name]

    def _body(*args):
        operands = list(args)
        if partition_name is not None:
            operands.append(bass2jax.partition_id_tensor())
        outs = _bass_exec_p.bind(
            *operands,
            out_avals=tuple(out_avals),
            in_names=tuple(in_names),
            out_names=tuple(out_names),
            lowering_input_output_aliases=(),
            sim_require_finite=True,
            sim_require_nnan=True,
            nc=nc,
        )
        return tuple(outs)

    devices = jax.devices()[:cores]
    mesh = Mesh(np.asarray(devices), ("core",))
    in_specs = (PartitionSpec("core"),) * (n_params + len(out_names))
    out_specs = (PartitionSpec("core"),) * len(out_names)
    fn = jax.jit(
        shard_map(_body, mesh=mesh, in_specs=in_specs, out_specs=out_specs,
                  check_rep=False),
        keep_unused=True,
    )
    sh = NamedSharding(mesh, PartitionSpec("core"))
    dev_in = [
        jax.device_put(
            np.concatenate([in_maps[c][nm] for c in range(cores)], axis=0), sh
        )
        for nm in in_names[:n_params]
    ]
    dev_zero = [
        jax.device_put(
            np.zeros((cores * z.shape[0], *z.shape[1:]), z.dtype), sh
        )
        for z in zero_outs
    ]
    out = fn(*dev_in, *dev_zero)
    jax.block_until_ready(out)
    times = []
    for _ in range(iters):
        t0 = time.perf_counter()
        out = fn(*dev_in, *dev_zero)
        jax.block_until_ready(out)
        times.append(time.perf_counter() - t0)
    out_np = np.asarray(out[0]).reshape(cores, *out_avals[0].shape)
    result = np.concatenate([out_np[c].T for c in range(cores)], axis=0)
    return result.astype(np.float32), times


# revision 3
# speedup vs baseline: 1054.3507x; 1.0042x over previous
"""Trainium2 Bass kernel for nn_Net_20512763805724 (dense_mlp, 3-layer SLP net).

Math (per layer, input p [B,L], weight w [O,L]):
    wb = sign(w)  (w>=0 -> +1 else -1)
    e  = 2p-1 ; d = 4p(1-p)
    out = (sum(d+e^2) + (e@wb.T)^2 - (e^2)@(wb^2).T) / L^2
Since d+e^2 == 1 exactly and wb^2 == 1:
    out[b,o] = (C[b] + s1[b,o]^2) / L^2
    s1 = 2*(p@wb.T) - c[o],  c[o] = sum_f wb[o,f]
    C[b] = 4*sum_f p(1-p)

Sharding: pure data parallel over batch, 8 cores x 8192 rows.  The host
pre-tiles + transposes each core's x-shard (feature-major, bf16) so on-chip
everything stays in "transposed" layout (features on SBUF partitions, batch
on the free dim).  All weight prep (binarize, column sums -> biases,
partition replication) is host-side so the device starts streaming x
immediately.  Output is outT [4, 8192] per core, un-transposed on the host.

Per pair of 512-batch tiles (bf16 data, fp32 PSUM), instruction-interleaved
so the two tiles' matmuls run CONCURRENTLY in different PE column groups
(tile_position (0,0) / (0,64)):
  - s1 matmuls: psumA[0:64]/[64:128] = wb1T.T @ xT  (6 k-chunks x 2 tiles)
  - C-term, split by chunk group:
      chunks 0..2: DVE computes sq = x*x (tensor_tensor, 2x bf16 mode);
        ones(+s) matmuls stream x, ones(-s) matmuls stream sq
        -> contributes +s*sum(x - x^2)
      chunks 3..5: ACT computes (x-0.5)^2 = 0.25 - q in one Square;
        ones(-s) matmuls stream it; the 0.25*128*3 constant is added
        back as K1 in the combine.
  - ACT: t = Square(psumA*(2/L) + bias) = s1^2/L^2 ; DVE: p2 = (t+K1)+psumB
B-matmuls + combine of pair p are emitted one pair later (software
pipeline skew) so DVE/ACT latency never stalls the PE stream.
Layer-2/3 psums are partition-stacked (2 tiles -> [128,512] for L2, 4 for
L3) via tile_position.  Output DMAs ride the gpsimd queue.
"""

import sys

if "/opt/trn_rl_repo" not in sys.path:
    sys.path.insert(0, "/opt/trn_rl_repo")

import ml_dtypes
import numpy as np

BF16 = ml_dtypes.bfloat16

B = 65536
IN_DIM = 768
NCORES = 8
BC = B // NCORES            # 8192 rows per core
TILE = 512                  # batch tile (max fp32 PSUM free dim)
NT = BC // TILE             # 16 tiles per core
NPAIR = NT // 2
NCHUNK = IN_DIM // 128      # 6 feature chunks for layer 1
L1, O1 = 768, 64
L2, O2 = 32 * 2, 32
L3, O3 = 32, 4
N_DVE_CHUNKS = 3            # layer-1 chunks 0..2: DVE sq + x/sq streams
N_ACT_CHUNKS = NCHUNK - N_DVE_CHUNKS   # chunks 3..5: ACT (x-0.5)^2 stream
XBUFS = 10                  # x-tile prefetch depth
# C-term scale: the ones lhsT holds +/-s in bf16, so use the bf16-rounded
# value as the effective scale everywhere it must stay consistent.
S1_EFF = float(np.float32(BF16(4.0 / (L1 * L1))))
S2 = 4.0 / (L2 * L2)        # 2^-10, exact in bf16
S3 = 4.0 / (L3 * L3)        # 2^-8, exact in bf16
# ACT chunks compute (x-0.5)^2 instead of (x^2-x); each contributes an extra
# -s*0.25*128 into psumB that we add back as a constant in the combine.
K1 = S1_EFF * 0.25 * 128 * N_ACT_CHUNKS

_CACHE = {}


def _build(reps=1):
    import contextlib

    import concourse.bacc as bacc
    import concourse.mybir as mybir
    import concourse.tile as tile

    f32 = mybir.dt.float32
    bf16 = mybir.dt.bfloat16
    AOP = mybir.AluOpType
    Square = mybir.ActivationFunctionType.Square

    nc = bacc.Bacc(None, target_bir_lowering=False)

    xt = nc.declare_dram_parameter("xt", [NT, 128, NCHUNK * TILE], bf16,
                                   isOutput=False)
    w1d = nc.declare_dram_parameter("w1d", [128, NCHUNK, O1], bf16,
                                    isOutput=False)
    w2d = nc.declare_dram_parameter("w2d", [128, O2], bf16, isOutput=False)
    w3d = nc.declare_dram_parameter("w3d", [128, O3], bf16, isOutput=False)
    b1d = nc.declare_dram_parameter("b1d", [128, 1], f32, isOutput=False)
    b2d = nc.declare_dram_parameter("b2d", [128, 1], f32, isOutput=False)
    b3d = nc.declare_dram_parameter("b3d", [128, 1], f32, isOutput=False)
    outt = nc.declare_dram_parameter("outt", [O3, BC], f32, isOutput=True)

    ND, NA = N_DVE_CHUNKS, N_ACT_CHUNKS

    with tile.TileContext(nc) as tc:
        with (
            tc.tile_pool(name="const", bufs=1) as cpool,
            tc.tile_pool(name="xp", bufs=XBUFS) as xpool,
            tc.tile_pool(name="sqp", bufs=4) as sqpool,
            tc.tile_pool(name="qap", bufs=4) as qapool,
            tc.tile_pool(name="sb", bufs=2) as spool,
            tc.tile_pool(name="psA", bufs=2, space="PSUM") as pA,
            tc.tile_pool(name="psB", bufs=2, space="PSUM") as pB,
            tc.tile_pool(name="psS", bufs=1, space="PSUM") as pS,
        ):
            # ------------- constants (host-precomputed weights) -------------
            lhsT1 = cpool.tile([128, NCHUNK, O1], bf16)
            nc.gpsimd.dma_start(out=lhsT1, in_=w1d[:])
            lhsT2 = cpool.tile([128, O2], bf16)
            nc.gpsimd.dma_start(out=lhsT2, in_=w2d[:])
            lhsT3 = cpool.tile([128, O3], bf16)
            nc.gpsimd.dma_start(out=lhsT3, in_=w3d[:])
            bias1 = cpool.tile([128, 1], f32)
            nc.gpsimd.dma_start(out=bias1, in_=b1d[:])
            bias2 = cpool.tile([128, 1], f32)
            nc.gpsimd.dma_start(out=bias2, in_=b2d[:])
            bias3 = cpool.tile([128, 1], f32)
            nc.gpsimd.dma_start(out=bias3, in_=b3d[:])

            onesP1 = cpool.tile([128, O1], bf16)
            nc.vector.memset(onesP1, S1_EFF)
            onesN1 = cpool.tile([128, O1], bf16)
            nc.vector.memset(onesN1, -S1_EFF)
            onesB2 = cpool.tile([128, O2], bf16)
            nc.vector.memset(onesB2, -S2)
            onesB3 = cpool.tile([128, O3], bf16)
            nc.vector.memset(onesB3, -S3)
            biasq = cpool.tile([128, 1], f32)
            nc.vector.memset(biasq, -0.5)

            # ---------------- main loop (1-pair software pipeline) ----------
            state = {}   # per-pair tiles carried to the tail stage

            def head(pr):
                """DMA + s1-matmuls + sq/q elementwise for pair pr."""
                t0, t1 = 2 * pr, 2 * pr + 1
                xs = []
                for tt in (t0, t1):
                    x_t = xpool.tile([128, NCHUNK * TILE], bf16, tag="x",
                                     name=f"x_{tt}")
                    nc.sync.dma_start(out=x_t, in_=xt[tt])
                    xs.append(x_t)
                psA1 = pA.tile([128, TILE], f32, tag="A1", name=f"psA1_{pr}")
                psB1 = pB.tile([128, TILE], f32, tag="B1", name=f"psB1_{pr}")
                # interleaved s1 matmuls: the two tiles alternate column
                # groups every instruction -> concurrent in the PE array
                for c in range(NCHUNK):
                    for k in range(2):
                        nc.tensor.matmul(
                            psA1[O1 * k : O1 * (k + 1), :],
                            lhsT1[:, c, :],
                            xs[k][:, c * TILE : (c + 1) * TILE],
                            start=(c == 0),
                            stop=(c == NCHUNK - 1),
                            tile_position=(0, O1 * k),
                        )
                sqs, qas = [], []
                for k, tt in enumerate((t0, t1)):
                    sq = sqpool.tile([128, ND * TILE], bf16, tag="sq",
                                     name=f"sq_{tt}")
                    nc.vector.tensor_mul(
                        sq, xs[k][:, 0 : ND * TILE], xs[k][:, 0 : ND * TILE]
                    )
                    sqs.append(sq)
                    qa = qapool.tile([128, NA * TILE], bf16, tag="qa",
                                     name=f"qa_{tt}")
                    nc.scalar.activation(
                        qa, xs[k][:, ND * TILE :], Square, bias=biasq,
                        scale=1.0,
                    )
                    qas.append(qa)
                state[pr] = (xs, sqs, qas, psA1, psB1)

            def tail(pr):
                """C-term matmuls + combine + layers 2/3 for pair pr."""
                t1 = 2 * pr + 1
                qj = t1 % 4
                xs, sqs, qas, psA1, psB1 = state.pop(pr)
                # C-term streams, interleaved across the two tiles:
                #   +s * x (chunks 0..2), -s * x^2, -s * (x-0.5)^2
                nmm = ND * 2 + NA
                i = 0
                for src_list, ones, base in (
                    (xs, onesP1, 0),
                    (sqs, onesN1, 0),
                    (qas, onesN1, 0),
                ):
                    nch = ND if src_list is not qas else NA
                    width = nch * TILE
                    for c in range(nch):
                        for k in range(2):
                            nc.tensor.matmul(
                                psB1[O1 * k : O1 * (k + 1), :],
                                ones,
                                src_list[k][:, c * TILE : (c + 1) * TILE],
                                start=(i == 0),
                                stop=(i == nmm - 1),
                                tile_position=(0, O1 * k),
                            )
                        i += 1
                # layer-1 combine for the pair -> p2 [128, 512]
                t2p = spool.tile([128, TILE], f32, tag="t2", name=f"t2_{pr}")
                nc.scalar.activation(
                    t2p, psA1, Square, bias=bias1, scale=2.0 / L1
                )
                p2p = spool.tile([128, TILE], bf16, tag="p2", name=f"p2_{pr}")
                nc.vector.scalar_tensor_tensor(
                    p2p, t2p, K1, psB1, AOP.add, AOP.add
                )
                # layer 2 for both tiles of the pair
                q2p = spool.tile([128, TILE], bf16, tag="q2", name=f"q2_{pr}")
                nc.vector.scalar_tensor_tensor(
                    q2p, p2p, 1.0, p2p, AOP.subtract, AOP.mult
                )
                if qj == 1:
                    state["psA2"] = pA.tile([128, TILE], f32, tag="A2",
                                            bufs=1, name=f"psA2_{pr}")
                    state["psB2"] = pB.tile([128, TILE], f32, tag="B2",
                                            bufs=1, name=f"psB2_{pr}")
                psA2, psB2 = state["psA2"], state["psB2"]
                for half, tq in ((0, qj - 1), (1, qj)):
                    hs = slice(O1 * half, O1 * (half + 1))
                    tp = (O1 * half, O2 * tq)
                    nc.tensor.matmul(
                        psA2[O2 * tq : O2 * (tq + 1), :],
                        lhsT2[hs, :],
                        p2p[hs, :],
                        tile_position=tp,
                    )
                    nc.tensor.matmul(
                        psB2[O2 * tq : O2 * (tq + 1), :],
                        onesB2[hs, :],
                        q2p[hs, :],
                        tile_position=tp,
                    )

                if qj == 3:
                    # layer-2 combine for the quad -> p3 [128, 512]
                    tq2 = spool.tile([128, TILE], f32, tag="tq2",
                                     name=f"tq2_{pr}")
                    nc.scalar.activation(
                        tq2, psA2, Square, bias=bias2, scale=2.0 / L2
                    )
                    p3q = spool.tile([128, TILE], bf16, tag="p3",
                                     name=f"p3_{pr}")
                    nc.vector.scalar_tensor_tensor(
                        p3q, tq2, 0.0, psB2, AOP.add, AOP.add
                    )
                    # layer 3
                    q3q = spool.tile([128, TILE], bf16, tag="q3",
                                     name=f"q3_{pr}")
                    nc.vector.scalar_tensor_tensor(
                        q3q, p3q, 1.0, p3q, AOP.subtract, AOP.mult
                    )
                    psA3 = pS.tile([128, TILE], f32, tag="A3",
                                   name=f"psA3_{pr}")
                    psB3 = pS.tile([128, TILE], f32, tag="B3",
                                   name=f"psB3_{pr}")
                    for j in range(4):
                        js = slice(32 * j, 32 * j + L3)
                        tp = (32 * j, 32 * j)
                        nc.tensor.matmul(
                            psA3[32 * j : 32 * j + O3, :],
                            lhsT3[js, :],
                            p3q[js, :],
                            tile_position=tp,
                        )
                        nc.tensor.matmul(
                            psB3[32 * j : 32 * j + O3, :],
                            onesB3[js, :],
                            q3q[js, :],
                            tile_position=tp,
                        )
                    t3q = spool.tile([128, TILE], f32, tag="t3",
                                     name=f"t3_{pr}")
                    nc.scalar.activation(
                        t3q, psA3, Square, bias=bias3, scale=2.0 / L3
                    )
                    outq = spool.tile([128, TILE], f32, tag="outq",
                                      name=f"outq_{pr}")
                    nc.vector.scalar_tensor_tensor(
                        outq, t3q, 0.0, psB3, AOP.add, AOP.add
                    )
                    for j in range(4):
                        tt = t1 - 3 + j
                        nc.gpsimd.dma_start(
                            out=outt[:, tt * TILE : (tt + 1) * TILE],
                            in_=outq[32 * j : 32 * j + O3, :],
                        )

            loop_cm = (
                tc.For_i(0, reps, 1) if reps > 1 else contextlib.nullcontext()
            )
            with loop_cm:
                for pr in range(NPAIR):
                    head(pr)
                    if pr >= 1:
                        tail(pr - 1)
                tail(NPAIR - 1)

    nc.compile()
    return nc


def _get_nc(reps=1):
    key = ("nc", reps)
    if key not in _CACHE:
        _CACHE[key] = _build(reps)
    return _CACHE[key]


def _make_in_maps(x, w1, w2, w3):
    x = np.asarray(x, dtype=np.float32)
    w1 = np.asarray(w1, dtype=np.float32)
    w2 = np.asarray(w2, dtype=np.float32)
    w3 = np.asarray(w3, dtype=np.float32)

    # binarized weights and their column sums (all tiny -> host)
    wb1 = np.where(w1 >= 0, 1.0, -1.0).astype(np.float32)   # [64, 768]
    wb2 = np.where(w2 >= 0, 1.0, -1.0).astype(np.float32)   # [32, 64]
    wb3 = np.where(w3 >= 0, 1.0, -1.0).astype(np.float32)   # [4, 32]

    # lhsT1[p, c, o] = wb1[o, c*128+p]
    w1d = np.ascontiguousarray(
        wb1.T.reshape(NCHUNK, 128, O1).transpose(1, 0, 2)
    ).astype(BF16)
    # lhsT2: [128, 32], rows 0..63 = wb2.T, rows 64..127 = copy
    w2d = np.ascontiguousarray(np.tile(wb2.T, (2, 1))).astype(BF16)
    # lhsT3: [128, 4], wb3.T replicated 4x
    w3d = np.ascontiguousarray(np.tile(wb3.T, (4, 1))).astype(BF16)

    c1 = wb1.sum(axis=1)   # [64]
    c2 = wb2.sum(axis=1)   # [32]
    c3 = wb3.sum(axis=1)   # [4]
    b1d = np.ascontiguousarray(
        np.tile(-c1 / L1, 2).reshape(128, 1)
    ).astype(np.float32)
    b2d = np.ascontiguousarray(
        np.tile(-c2 / L2, 4).reshape(128, 1)
    ).astype(np.float32)
    b3 = np.zeros((4, 32), np.float32)
    b3[:, :O3] = -c3 / L3
    b3d = np.ascontiguousarray(b3.reshape(128, 1))

    xs = x.reshape(NCORES, NT, TILE, NCHUNK, 128)
    # [core][tile, partition(f%128), chunk(f//128)*TILE + batch-in-tile], bf16
    xtiled = np.ascontiguousarray(
        xs.transpose(0, 1, 4, 3, 2).astype(BF16)
    ).reshape(NCORES, NT, 128, NCHUNK * TILE)
    return [
        {
            "xt": xtiled[i],
            "w1d": w1d,
            "w2d": w2d,
            "w3d": w3d,
            "b1d": b1d,
            "b2d": b2d,
            "b3d": b3d,
        }
        for i in range(NCORES)
    ]


def kernel(x, w1, w2, w3):
    from concourse.bass_utils import run_bass_kernel_spmd

    nc = _get_nc()
    in_maps = _make_in_maps(x, w1, w2, w3)
    res = run_bass_kernel_spmd(nc, in_maps, core_ids=list(range(NCORES)))
    return np.concatenate(
        [res.results[i]["outt"].T for i in range(NCORES)], axis=0
    ).astype(np.float32)


def bench(x, w1, w2, w3, iters=20, reps=1, cores=NCORES):
    """Time device execution with a persistent jit and device-resident
    inputs (excludes host<->device transfer and compile).  Returns
    (output, per_call_seconds_list).  NOTE: per-call wall time under axon
    is dominated by a fixed ~80ms relay dispatch latency; use the NTFF
    profile (run_bass_kernel_spmd(trace=True)) for true HW exec time."""
    import time

    import jax
    from jax.sharding import Mesh, NamedSharding, PartitionSpec
    from jax.experimental.shard_map import shard_map

    import concourse.mybir as mybir
    from concourse import bass2jax
    from concourse.bass2jax import _bass_exec_p, install_neuronx_cc_hook

    nc = _get_nc(reps)
    install_neuronx_cc_hook()
    in_maps = _make_in_maps(x, w1, w2, w3)

    partition_name = (
        nc.partition_id_tensor.name if nc.partition_id_tensor else None
    )
    in_names, out_names, out_avals, zero_outs = [], [], [], []
    for alloc in nc.m.functions[0].allocations:
        if not isinstance(alloc, mybir.MemoryLocationSet):
            continue
        name = alloc.memorylocations[0].name
        if alloc.kind == "ExternalInput":
            if name != partition_name:
                in_names.append(name)
        elif alloc.kind == "ExternalOutput":
            out_names.append(name)
            shape = tuple(alloc.tensor_shape)
            dtype = mybir.dt.np(alloc.dtype)
            out_avals.append(jax.core.ShapedArray(shape, dtype))
            zero_outs.append(np.zeros(shape, dtype))
    n_params = len(in_names)
    in_names = in_names + out_names
    if partition_name is not None:
        in_names = in_names + [partition_name]

    def _body(*args):
        operands = list(args)
        if partition_name is not None:
            operands.append(bass2jax.partition_id_tensor())
        outs = _bass_exec_p.bind(
            *operands,
            out_avals=tuple(out_avals),
            in_names=tuple(in_names),
            out_names=tuple(out_names),
            lowering_input_output_aliases=(),
            sim_require_finite=True,
            sim_require_nnan=True,
            nc=nc,
        )
        return tuple(outs)

    devices = jax.devices()[:cores]
    mesh = Mesh(np.asarray(devices), ("core",))
    in_specs = (PartitionSpec("core"),) * (n_params + len(out_names))
    out_specs = (PartitionSpec("core"),) * len(out_names)
    fn = jax.jit(
        shard_map(_body, mesh=mesh, in_specs=in_specs, out_specs=out_specs,
                  check_rep=False),
        keep_unused=True,
    )
    sh = NamedSharding(mesh, PartitionSpec("core"))
    dev_in = [
        jax.device_put(
            np.concatenate([in_maps[c][nm] for c in range(cores)], axis=0), sh
        )
        for nm in in_names[:n_params]
    ]
    dev_zero = [
        jax.device_put(
            np.zeros((cores * z.shape[0], *z.shape[1:]), z.dtype), sh
        )
        for z in zero_outs
    ]
    out = fn(*dev_in, *dev_zero)
    jax.block_until_ready(out)
    times = []
    for _ in range(iters):
        t0 = time.perf_counter()
        out = fn(*dev_in, *dev_zero)
        jax.block_until_ready(out)
        times.append(time.perf_counter() - t0)
    out_np = np.asarray(out[0]).reshape(cores, *out_avals[0].shape)
    result = np.concatenate([out_np[c].T for c in range(cores)], axis=0)
    return result.astype(np.float32), times
